# revision 6
# baseline (speedup 1.0000x reference)
"""Trainium2 Bass kernel for nn_ChannelMerger (v7).

Computation (per batch b):
    emb   = fourier_emb(positions[b])            # [C, D]   D=288  (host)
    scores= emb @ heads.T                        # [C, O]   O=270  (device)
    w     = softmax(scores + mask_offset, axis=C)           (device exp/sums)
    out[b]= (w.T @ meg[b])                       # [O, T]   (device)

Sharding: data-parallel over batch B=32 across 8 cores (4 batches/core).

Design (v6, trace-driven):
  - fourier embedding computed on host; embT uploaded bf16.  Removes
    the Sin activation so the kernel needs ONE activation table set
    (exp_and_others: Exp + Copy), pre-triggered at t=0 by a dummy Exp.
  - C contraction chunks 128/128/17: the bulk meg transfer is a
    [128, 2, T] tile spanning all 16 SBUF DMA ports (a 96-row tile
    only reaches 12), with a tiny [17, T] remainder.
  - emission interleaved per batch (weights(b+1) after big(b)) so only
    cst1 + meg[0] gate the start of the big-matmul stream; cst2 and
    later meg loads hide under big(0..2).
  - cst1 and meg[0] ride the SWDGE (gpsimd) queue, which reaches line
    rate several us earlier than the scalar HWDGE ring; meg[0] is
    loaded in T-halves so big(0) starts after the first half.
  - a 10-matmul dummy burst on a scratch tile warms the PE HAM clock
    gate (K=8/8) during the otherwise-dead DMA lead-in; without it the
    interleaved stream never reaches a fully-busy 3.4us HAM window and
    the whole kernel runs at 1.2 GHz.
  - sume matmuls emitted one batch late so the ACT Exp chain never
    stalls the PE FIFO; 1/sum applied on host.
  - ALL output stores on the (otherwise untouched) sync ring.

Output dram layout is [BPC, 128, 32*270] bf16 with out[b, t, o] at
[b, t % 128, (t // 128)*270 + o]; host untangles, upcasts, divides by
the softmax sums.
"""

import math

import numpy as np

import concourse.bacc as bacc
import concourse.bass as bass
import concourse.mybir as mybir
from concourse.bass_utils import run_bass_kernel_spmd
from concourse.tile import TileContext

# Problem shape (hardcoded per contract)
B, C, T = 32, 273, 4096
O, D = 270, 288
NF = 12            # fourier freqs per axis (sqrt(D/2))
MARGIN = 0.1
NCORES = 8
BPC = B // NCORES  # batches per core

C0S = [0, 128, 256]          # C contraction chunk starts
CWS = [128, 128, 17]         # C contraction chunk widths (sum = 273)
DK = 96                      # D chunk width (3 chunks of 96 = 288)
CP = 274                     # C padded to even for the embT layout

TCH = 128          # T chunk = psum partition dim of the big matmul
NTH = T // TCH     # 32
OW = NTH * O       # out staging columns per partition (8640)

NEG_BIG = -1.0e30  # stands in for -inf on masked channels

# cst1 ([96, CW1] bf16): headsT chunks, then embT(b=0)
HD_C0 = 0
EMB0_C0 = 3 * O
CW1 = EMB0_C0 + 3 * CP
# cst2 ([96, CW2] bf16): embT(b=1..3)
CW2 = 3 * 3 * CP

F32 = mybir.dt.float32
BF16 = mybir.dt.bfloat16

_CACHE = {}
LAST_RESULTS = None         # BassKernelResults of the most recent run (for test.py)


def _host_emb(positions):
    """fourier_emb on host: positions [B, C, 2] f32 -> emb [B, C, D] f64."""
    p = (2.0 * math.pi / (1.0 + 2.0 * MARGIN)) * np.arange(NF, dtype=np.float64)
    pos = positions.astype(np.float64) + MARGIN
    loc = (
        pos[..., 0, None, None] * p[:, None] + pos[..., 1, None, None] * p[None, :]
    ).reshape(*positions.shape[:-1], NF * NF)
    return np.concatenate([np.cos(loc), np.sin(loc)], axis=-1)


def _build_program():
    nc = bacc.Bacc(
        trn_type="TRN2",
        target_bir_lowering=False,
        debug=False,
        dynamic_dma_scratch_size=32768,
    )

    # meg pre-chunked on host: megC01[b, p, j, t] = meg[b, j*128 + p, t],
    # megC2[b, p, t] = meg[b, 256 + p, t].
    megC01 = nc.dram_tensor("megC01", [BPC, 128, 2, T], BF16, kind="ExternalInput").ap()
    megC2 = nc.dram_tensor("megC2", [BPC, 17, T], BF16, kind="ExternalInput").ap()
    cst1d = nc.dram_tensor("cst1d", [96, CW1], BF16, kind="ExternalInput").ap()
    cst2d = nc.dram_tensor("cst2d", [96, CW2], BF16, kind="ExternalInput").ap()
    cstOd = nc.dram_tensor("cstOd", [128, 3 * BPC], F32, kind="ExternalInput").ap()
    out = nc.dram_tensor("out", [BPC, TCH, OW], BF16, kind="ExternalOutput").ap()
    sumd = nc.dram_tensor("sumd", [1, BPC * O], F32, kind="ExternalOutput").ap()

    with TileContext(nc) as tc:
        with (
            tc.tile_pool(name="singles", bufs=1) as singles,
            tc.tile_pool(name="w", bufs=2) as wp,
            tc.tile_pool(name="megp", bufs=4) as megp,
            tc.tile_pool(name="outp", bufs=2) as outp,
            tc.tile_pool(name="psc", bufs=2, space="PSUM") as psc,
            tc.tile_pool(name="pss", bufs=1, space="PSUM") as pss,
            tc.tile_pool(name="psbig", bufs=5, space="PSUM") as psbig,
        ):
            # ---- dummy Exp first: pulls the exp_and_others ACT table load
            # to t=0 (it is the only table set the kernel ever needs)
            ones_sb = singles.tile([128, 1], BF16, name="ones_sb")
            nc.vector.memset(ones_sb, 1.0)
            scratch = singles.tile([1, 1], F32, name="scratch")
            nc.scalar.activation(
                scratch, ones_sb[0:1, 0:1], mybir.ActivationFunctionType.Exp
            )

            # ---- HAM warm-up: dense dummy matmul burst on scratch data ----
            warm_sb = singles.tile([128, 512], BF16, name="warm_sb")
            nc.vector.memset(warm_sb, 0.0)
            warm_ps = psc.tile([128, 512], F32, name="warm_ps", tag="sc")
            for i in range(10):
                nc.tensor.matmul(
                    warm_ps,
                    warm_sb[:, 0:128],
                    warm_sb,
                    start=(i == 0),
                    stop=(i == 9),
                )

            # ---- const loads: cst1 on the SWDGE queue (fast early) ----
            cst1 = singles.tile([96, CW1], BF16, name="cst1")
            nc.gpsimd.dma_start(out=cst1, in_=cst1d)
            cstO = singles.tile([128, 3 * BPC], F32, name="cstO")
            nc.scalar.dma_start(out=cstO, in_=cstOd)

            headsT = [
                cst1[:, HD_C0 + k * O : HD_C0 + (k + 1) * O] for k in range(3)
            ]

            # ---- meg loads ----
            megs = {}
            HT = T // 2

            def load_meg(b, eng, half=False):
                mA = megp.tile([128, 2, T], BF16, name=f"megA_b{b}", tag="megA")
                mB = megp.tile([17, T], BF16, name=f"megB_b{b}", tag="megB")
                if half:
                    eng.dma_start(out=mA[:, :, 0:HT], in_=megC01[b, :, :, 0:HT])
                    eng.dma_start(out=mB, in_=megC2[b])
                    eng.dma_start(out=mA[:, :, HT:], in_=megC01[b, :, :, HT:])
                else:
                    eng.dma_start(out=mA, in_=megC01[b])
                    eng.dma_start(out=mB, in_=megC2[b])
                megs[b] = (mA, mB)

            load_meg(0, nc.gpsimd, half=True)

            cst2 = singles.tile([96, CW2], BF16, name="cst2")
            nc.scalar.dma_start(out=cst2, in_=cst2d)

            # b1 split across both load queues; b2 scalar; b3 gpsimd
            m1A = megp.tile([128, 2, T], BF16, name="megA_b1", tag="megA")
            m1B = megp.tile([17, T], BF16, name="megB_b1", tag="megB")
            nc.gpsimd.dma_start(out=m1A, in_=megC01[1])
            nc.scalar.dma_start(out=m1B, in_=megC2[1])
            megs[1] = (m1A, m1B)
            load_meg(2, nc.scalar)
            load_meg(3, nc.gpsimd)

            def embT(b, k):
                if b == 0:
                    return cst1[:, EMB0_C0 + k * CP : EMB0_C0 + k * CP + CP]
                i = (b - 1) * 3 + k
                return cst2[:, i * CP : i * CP + CP]

            sume_sb = singles.tile([1, BPC * O], F32, name="sume_sb")

            expT = {}

            # ---- weights: scores matmuls + Exp for one batch ----
            def weights(b):
                for j in range(3):
                    c0, cw = C0S[j], CWS[j]
                    sc = psc.tile([128, O], F32, name=f"sc_b{b}j{j}", tag="sc")
                    for k in range(3):
                        nc.tensor.matmul(
                            sc[0:cw, :],
                            embT(b, k)[:, c0 : c0 + cw],
                            headsT[k],
                            start=(k == 0),
                            stop=(k == 2),
                        )
                    ex = wp.tile([128, O], BF16, name=f"expT_b{b}j{j}", tag=f"expT{j}")
                    nc.scalar.activation(
                        ex[0:cw, :],
                        sc[0:cw, :],
                        mybir.ActivationFunctionType.Exp,
                        bias=cstO[0:cw, b * 3 + j : b * 3 + j + 1],
                    )
                    expT[(b, j)] = ex

            def w_sume(b):
                sume = pss.tile([1, O], F32, name=f"sume_b{b}", tag="sume")
                for j in range(3):
                    cw = CWS[j]
                    nc.tensor.matmul(
                        sume,
                        ones_sb[0:cw, :],
                        expT[(b, j)][0:cw, :],
                        start=(j == 0),
                        stop=(j == 2),
                    )
                nc.vector.tensor_copy(out=sume_sb[:, b * O : (b + 1) * O], in_=sume)

            # ---- big matmuls ----
            def big_matmul(b):
                mA, mB = megs[b]
                ob = outp.tile([TCH, OW], BF16, name=f"out_b{b}", tag="out")
                nparts = 2 if b + 1 < BPC else 8
                step = OW // nparts
                for th in range(NTH):
                    pb = psbig.tile([TCH, O], F32, name=f"pb_b{b}t{th}", tag="pb")
                    for j in range(3):
                        cw = CWS[j]
                        if j < 2:
                            lhsT = mA[:, j, th * TCH : (th + 1) * TCH]
                        else:
                            lhsT = mB[:, th * TCH : (th + 1) * TCH]
                        nc.tensor.matmul(
                            pb,
                            lhsT,
                            expT[(b, j)][0:cw, :],
                            start=(j == 0),
                            stop=(j == 2),
                        )
                    dst = ob[:, th * O : (th + 1) * O]
                    if th % 2 == 0:
                        nc.vector.tensor_copy(out=dst, in_=pb)
                    else:
                        nc.scalar.activation(
                            dst, pb, mybir.ActivationFunctionType.Copy
                        )
                    done = (th + 1) * O
                    if done % step == 0:
                        q = done // step - 1
                        nc.sync.dma_start(
                            out=out[b, :, q * step : (q + 1) * step],
                            in_=ob[:, q * step : (q + 1) * step],
                        )

            weights(0)
            big_matmul(0)
            weights(1)
            w_sume(0)
            big_matmul(1)
            weights(2)
            w_sume(1)
            big_matmul(2)
            weights(3)
            w_sume(2)
            w_sume(3)
            nc.sync.dma_start(out=sumd, in_=sume_sb)
            big_matmul(3)
    nc.compile()
    return nc


def _get_program():
    if "nc" not in _CACHE:
        _CACHE["nc"] = _build_program()
    return _CACHE["nc"]


def kernel(meg, positions, heads, invalid_mask, trace=False):
    global LAST_RESULTS
    bf16 = mybir.dt.np(BF16)
    meg = np.asarray(meg, dtype=np.float32)
    positions = np.asarray(positions, dtype=np.float32)
    heads = np.asarray(heads, dtype=np.float32)

    megb = meg.astype(bf16)                                      # [B, C, T] bf16
    megC01 = np.ascontiguousarray(
        megb[:, 0:256, :].reshape(B, 2, 128, T).transpose(0, 2, 1, 3)
    )                                                            # [B, 128, 2, T]
    megC2 = np.ascontiguousarray(megb[:, 256:273, :])            # [B, 17, T]

    emb = _host_emb(positions)                                   # [B, C, D] f64
    headsT = heads.T                                             # [D, O]

    cst1 = np.zeros((NCORES, 96, CW1), bf16)
    cst2 = np.zeros((NCORES, 96, CW2), bf16)
    cstO = np.zeros((NCORES, 128, 3 * BPC), np.float32)
    for k in range(3):
        cst1[:, :, HD_C0 + k * O : HD_C0 + (k + 1) * O] = headsT[
            k * DK : (k + 1) * DK, :
        ].astype(bf16)

    maskf = np.asarray(invalid_mask, dtype=bool)                 # [B, C]
    for cix in range(NCORES):
        for bl in range(BPC):
            bg = cix * BPC + bl
            # embT(b,k)[d, c] = emb[bg, c, k*96 + d]
            eT = emb[bg].T.astype(bf16)                          # [D, C]
            for k in range(3):
                blk = eT[k * DK : (k + 1) * DK, :]               # [96, C]
                if bl == 0:
                    cst1[cix, :, EMB0_C0 + k * CP : EMB0_C0 + k * CP + C] = blk
                else:
                    i = (bl - 1) * 3 + k
                    cst2[cix, :, i * CP : i * CP + C] = blk
            for j in range(3):
                c0, cw = C0S[j], CWS[j]
                m = maskf[bg, c0 : c0 + cw].astype(np.float32) * NEG_BIG
                cstO[cix, 0:cw, bl * 3 + j] = m

    nc = _get_program()
    in_maps = []
    for cix in range(NCORES):
        s = slice(cix * BPC, (cix + 1) * BPC)
        in_maps.append(
            {
                "megC01": np.ascontiguousarray(megC01[s]),
                "megC2": np.ascontiguousarray(megC2[s]),
                "cst1d": np.ascontiguousarray(cst1[cix]),
                "cst2d": np.ascontiguousarray(cst2[cix]),
                "cstOd": np.ascontiguousarray(cstO[cix]),
            }
        )

    res = run_bass_kernel_spmd(nc, in_maps, core_ids=list(range(NCORES)), trace=trace)
    LAST_RESULTS = res
    # out[b, t, o] lives at [b, t % 128, (t // 128)*270 + o], unnormalized
    raw = np.concatenate([r["out"] for r in res.results], axis=0)  # [B,128,OW]
    sume = np.concatenate(
        [r["sumd"].reshape(BPC, O) for r in res.results], axis=0
    )  # [B, O]
    full = raw.astype(np.float32).reshape(B, TCH, NTH, O) / sume[:, None, None, :]
    return np.ascontiguousarray(full.transpose(0, 3, 2, 1).reshape(B, O, T))


# revision 7
# speedup vs baseline: 1.0903x; 1.0903x over previous
"""Trainium2 Bass kernel for nn_ChannelMerger (v8).

Computation (per batch b):
    emb   = fourier_emb(positions[b])            # [C, D]   D=288  (host)
    scores= emb @ heads.T                        # [C, O]   O=270  (device)
    w     = softmax(scores + mask_offset, axis=C)           (device exp/sums)
    out[b]= (w.T @ meg[b])                       # [O, T]   (device)

Sharding: data-parallel over batch B=32 across 8 cores (4 batches/core).

Design (v8, trace-driven):
  - fourier embedding computed on host; embT uploaded bf16.  Removes
    the Sin activation so the kernel needs ONE activation table set
    (exp_and_others: Exp + Copy), pre-triggered at t=0 by a dummy Exp.
  - C contraction chunks 128/128/17: the bulk meg transfer is a
    [128, 2, T] tile spanning all 16 SBUF DMA ports (a 96-row tile
    only reaches 12), with a tiny [17, T] remainder.
  - emission interleaved per batch (weights(b+1) after big(b)) so only
    cst1 + meg[0] gate the start of the big-matmul stream; cst2 and
    later meg loads hide under big(0..2).
  - each load queue sustains only ~130 GB/s (~270 combined), so meg
    batches are split in T-halves interleaved across the gpsimd and
    scalar queues, ordered by deadline (big(b) start time).
  - a dummy-matmul warm-up burst flips the PE HAM clock gate (K=8/8)
    during the otherwise-dead DMA lead-in, and a second bridge burst
    (ending with matmuls gated on meg[0]'s first half) keeps the PE
    from idling >3.4us before big(0) — an idle MID window would
    re-throttle the PE to 1.2 GHz, a self-sustaining ~1.7x slowdown.
  - sume matmuls emitted one batch late so the ACT Exp chain never
    stalls the PE FIFO; 1/sum applied on host.
  - ALL output stores on the (otherwise untouched) sync ring.

Output dram layout is [BPC, 128, 32*270] bf16 with out[b, t, o] at
[b, t % 128, (t // 128)*270 + o]; host untangles, upcasts, divides by
the softmax sums.
"""

import math

import numpy as np

import concourse.bacc as bacc
import concourse.bass as bass
import concourse.mybir as mybir
from concourse.bass_utils import run_bass_kernel_spmd
from concourse.tile import TileContext

# Problem shape (hardcoded per contract)
B, C, T = 32, 273, 4096
O, D = 270, 288
NF = 12            # fourier freqs per axis (sqrt(D/2))
MARGIN = 0.1
NCORES = 8
BPC = B // NCORES  # batches per core

C0S = [0, 128, 256]          # C contraction chunk starts
CWS = [128, 128, 17]         # C contraction chunk widths (sum = 273)
DK = 96                      # D chunk width (3 chunks of 96 = 288)
CP = 274                     # C padded to even for the embT layout

TCH = 128          # T chunk = psum partition dim of the big matmul
NTH = T // TCH     # 32
OW = NTH * O       # out staging columns per partition (8640)

NEG_BIG = -1.0e30  # stands in for -inf on masked channels

# cst1 ([96, CW1] bf16): headsT chunks, then embT(b=0)
HD_C0 = 0
EMB0_C0 = 3 * O
CW1 = EMB0_C0 + 3 * CP
# cst2 ([96, CW2] bf16): embT(b=1..3)
CW2 = 3 * 3 * CP

F32 = mybir.dt.float32
BF16 = mybir.dt.bfloat16

_CACHE = {}
LAST_RESULTS = None         # BassKernelResults of the most recent run (for test.py)


def _host_emb(positions):
    """fourier_emb on host: positions [B, C, 2] f32 -> emb [B, C, D] f64."""
    p = (2.0 * math.pi / (1.0 + 2.0 * MARGIN)) * np.arange(NF, dtype=np.float64)
    pos = positions.astype(np.float64) + MARGIN
    loc = (
        pos[..., 0, None, None] * p[:, None] + pos[..., 1, None, None] * p[None, :]
    ).reshape(*positions.shape[:-1], NF * NF)
    return np.concatenate([np.cos(loc), np.sin(loc)], axis=-1)


def _build_program():
    nc = bacc.Bacc(
        trn_type="TRN2",
        target_bir_lowering=False,
        debug=False,
        dynamic_dma_scratch_size=32768,
    )

    # meg pre-chunked on host: megC01[b, p, j, t] = meg[b, j*128 + p, t],
    # megC2[b, p, t] = meg[b, 256 + p, t].
    megC01 = nc.dram_tensor("megC01", [BPC, 128, 2, T], BF16, kind="ExternalInput").ap()
    megC2 = nc.dram_tensor("megC2", [BPC, 17, T], BF16, kind="ExternalInput").ap()
    cst1d = nc.dram_tensor("cst1d", [96, CW1], BF16, kind="ExternalInput").ap()
    cst2d = nc.dram_tensor("cst2d", [96, CW2], BF16, kind="ExternalInput").ap()
    cstOd = nc.dram_tensor("cstOd", [128, 3 * BPC], F32, kind="ExternalInput").ap()
    out = nc.dram_tensor("out", [BPC, TCH, OW], BF16, kind="ExternalOutput").ap()
    sumd = nc.dram_tensor("sumd", [1, BPC * O], F32, kind="ExternalOutput").ap()

    with TileContext(nc) as tc:
        with (
            tc.tile_pool(name="singles", bufs=1) as singles,
            tc.tile_pool(name="w", bufs=2) as wp,
            tc.tile_pool(name="megp", bufs=4) as megp,
            tc.tile_pool(name="outp", bufs=3) as outp,
            tc.tile_pool(name="psc", bufs=2, space="PSUM") as psc,
            tc.tile_pool(name="pss", bufs=1, space="PSUM") as pss,
            tc.tile_pool(name="psbig", bufs=5, space="PSUM") as psbig,
        ):
            # ---- dummy Exp first: pulls the exp_and_others ACT table load
            # to t=0 (it is the only table set the kernel ever needs)
            ones_sb = singles.tile([128, 1], BF16, name="ones_sb")
            nc.vector.memset(ones_sb, 1.0)
            scratch = singles.tile([1, 1], F32, name="scratch")
            nc.scalar.activation(
                scratch, ones_sb[0:1, 0:1], mybir.ActivationFunctionType.Exp
            )

            # ---- HAM warm-up: dense dummy matmul burst on scratch data ----
            warm_sb = singles.tile([128, 512], BF16, name="warm_sb")
            nc.vector.memset(warm_sb, 0.0)
            warm_ps = psc.tile([128, 512], F32, name="warm_ps", tag="sc")
            for i in range(10):
                nc.tensor.matmul(
                    warm_ps,
                    warm_sb[:, 0:128],
                    warm_sb,
                    start=(i == 0),
                    stop=(i == 9),
                )

            # ---- const loads: cst1 first on the SWDGE queue (fast early),
            # csts for later batches lead the scalar ring
            cst1 = singles.tile([96, CW1], BF16, name="cst1")
            nc.gpsimd.dma_start(out=cst1, in_=cst1d)
            cstO = singles.tile([128, 3 * BPC], F32, name="cstO")
            nc.scalar.dma_start(out=cstO, in_=cstOd)
            cst2 = singles.tile([96, CW2], BF16, name="cst2")
            nc.scalar.dma_start(out=cst2, in_=cst2d)

            headsT = [
                cst1[:, HD_C0 + k * O : HD_C0 + (k + 1) * O] for k in range(3)
            ]

            # ---- meg loads: halves interleaved across both queues in
            # deadline order (gpsimd: lower half + remainder; scalar:
            # upper half behind the csts)
            megs = {}
            HT = T // 2
            for b in range(BPC):
                mA = megp.tile([128, 2, T], BF16, name=f"megA_b{b}", tag="megA")
                mB = megp.tile([17, T], BF16, name=f"megB_b{b}", tag="megB")
                nc.gpsimd.dma_start(out=mA[:, :, 0:HT], in_=megC01[b, :, :, 0:HT])
                nc.gpsimd.dma_start(out=mB, in_=megC2[b])
                nc.scalar.dma_start(out=mA[:, :, HT:], in_=megC01[b, :, :, HT:])
                megs[b] = (mA, mB)

            def embT(b, k):
                if b == 0:
                    return cst1[:, EMB0_C0 + k * CP : EMB0_C0 + k * CP + CP]
                i = (b - 1) * 3 + k
                return cst2[:, i * CP : i * CP + CP]

            sume_sb = singles.tile([1, BPC * O], F32, name="sume_sb")

            expT = {}

            # ---- weights: scores matmuls + Exp for one batch ----
            def weights(b):
                for j in range(3):
                    c0, cw = C0S[j], CWS[j]
                    sc = psc.tile([128, O], F32, name=f"sc_b{b}j{j}", tag="sc")
                    for k in range(3):
                        nc.tensor.matmul(
                            sc[0:cw, :],
                            embT(b, k)[:, c0 : c0 + cw],
                            headsT[k],
                            start=(k == 0),
                            stop=(k == 2),
                        )
                    ex = wp.tile([128, O], BF16, name=f"expT_b{b}j{j}", tag=f"expT{j}")
                    nc.scalar.activation(
                        ex[0:cw, :],
                        sc[0:cw, :],
                        mybir.ActivationFunctionType.Exp,
                        bias=cstO[0:cw, b * 3 + j : b * 3 + j + 1],
                    )
                    expT[(b, j)] = ex

            def w_sume(b):
                sume = pss.tile([1, O], F32, name=f"sume_b{b}", tag="sume")
                for j in range(3):
                    cw = CWS[j]
                    nc.tensor.matmul(
                        sume,
                        ones_sb[0:cw, :],
                        expT[(b, j)][0:cw, :],
                        start=(j == 0),
                        stop=(j == 2),
                    )
                nc.vector.tensor_copy(out=sume_sb[:, b * O : (b + 1) * O], in_=sume)

            # ---- big matmuls ----
            def big_matmul(b):
                mA, mB = megs[b]
                ob = outp.tile([TCH, OW], BF16, name=f"out_b{b}", tag="out")
                nparts = 2 if b + 1 < BPC else 8
                step = OW // nparts
                for th in range(NTH):
                    pb = psbig.tile([TCH, O], F32, name=f"pb_b{b}t{th}", tag="pb")
                    for j in range(3):
                        cw = CWS[j]
                        if j < 2:
                            lhsT = mA[:, j, th * TCH : (th + 1) * TCH]
                        else:
                            lhsT = mB[:, th * TCH : (th + 1) * TCH]
                        nc.tensor.matmul(
                            pb,
                            lhsT,
                            expT[(b, j)][0:cw, :],
                            start=(j == 0),
                            stop=(j == 2),
                        )
                    dst = ob[:, th * O : (th + 1) * O]
                    if th % 2 == 0:
                        nc.vector.tensor_copy(out=dst, in_=pb)
                    else:
                        nc.scalar.activation(
                            dst, pb, mybir.ActivationFunctionType.Copy
                        )
                    done = (th + 1) * O
                    if done % step == 0:
                        q = done // step - 1
                        eng = nc.sync if (b < 2 or (b + q) % 2 == 0) else nc.scalar
                        eng.dma_start(
                            out=out[b, :, q * step : (q + 1) * step],
                            in_=ob[:, q * step : (q + 1) * step],
                        )

            weights(0)
            # ---- HAM bridge: keep the PE busy between scores(0) and
            # big(0).  The tail matmuls read meg[0]'s first half so the
            # bridge extends until big(0)'s own data is resident.
            mA0, _ = megs[0]
            bridge_ps = psbig.tile([128, 256], F32, name="bridge_ps", tag="pb")
            NBR = 30
            for i in range(NBR):
                rhs = warm_sb[:, 0:256] if i < NBR - 6 else mA0[:, 0, 0:256]
                nc.tensor.matmul(
                    bridge_ps,
                    warm_sb[:, 0:128],
                    rhs,
                    start=(i == 0),
                    stop=(i == NBR - 1),
                )
            big_matmul(0)
            weights(1)
            w_sume(0)
            big_matmul(1)
            weights(2)
            w_sume(1)
            big_matmul(2)
            weights(3)
            w_sume(2)
            w_sume(3)
            nc.sync.dma_start(out=sumd, in_=sume_sb)
            big_matmul(3)
    nc.compile()
    return nc


def _get_program():
    if "nc" not in _CACHE:
        _CACHE["nc"] = _build_program()
    return _CACHE["nc"]


def kernel(meg, positions, heads, invalid_mask, trace=False):
    global LAST_RESULTS
    bf16 = mybir.dt.np(BF16)
    meg = np.asarray(meg, dtype=np.float32)
    positions = np.asarray(positions, dtype=np.float32)
    heads = np.asarray(heads, dtype=np.float32)

    megb = meg.astype(bf16)                                      # [B, C, T] bf16
    megC01 = np.ascontiguousarray(
        megb[:, 0:256, :].reshape(B, 2, 128, T).transpose(0, 2, 1, 3)
    )                                                            # [B, 128, 2, T]
    megC2 = np.ascontiguousarray(megb[:, 256:273, :])            # [B, 17, T]

    emb = _host_emb(positions)                                   # [B, C, D] f64
    headsT = heads.T                                             # [D, O]

    cst1 = np.zeros((NCORES, 96, CW1), bf16)
    cst2 = np.zeros((NCORES, 96, CW2), bf16)
    cstO = np.zeros((NCORES, 128, 3 * BPC), np.float32)
    for k in range(3):
        cst1[:, :, HD_C0 + k * O : HD_C0 + (k + 1) * O] = headsT[
            k * DK : (k + 1) * DK, :
        ].astype(bf16)

    maskf = np.asarray(invalid_mask, dtype=bool)                 # [B, C]
    for cix in range(NCORES):
        for bl in range(BPC):
            bg = cix * BPC + bl
            # embT(b,k)[d, c] = emb[bg, c, k*96 + d]
            eT = emb[bg].T.astype(bf16)                          # [D, C]
            for k in range(3):
                blk = eT[k * DK : (k + 1) * DK, :]               # [96, C]
                if bl == 0:
                    cst1[cix, :, EMB0_C0 + k * CP : EMB0_C0 + k * CP + C] = blk
                else:
                    i = (bl - 1) * 3 + k
                    cst2[cix, :, i * CP : i * CP + C] = blk
            for j in range(3):
                c0, cw = C0S[j], CWS[j]
                m = maskf[bg, c0 : c0 + cw].astype(np.float32) * NEG_BIG
                cstO[cix, 0:cw, bl * 3 + j] = m

    nc = _get_program()
    in_maps = []
    for cix in range(NCORES):
        s = slice(cix * BPC, (cix + 1) * BPC)
        in_maps.append(
            {
                "megC01": np.ascontiguousarray(megC01[s]),
                "megC2": np.ascontiguousarray(megC2[s]),
                "cst1d": np.ascontiguousarray(cst1[cix]),
                "cst2d": np.ascontiguousarray(cst2[cix]),
                "cstOd": np.ascontiguousarray(cstO[cix]),
            }
        )

    res = run_bass_kernel_spmd(nc, in_maps, core_ids=list(range(NCORES)), trace=trace)
    LAST_RESULTS = res
    # out[b, t, o] lives at [b, t % 128, (t // 128)*270 + o], unnormalized
    raw = np.concatenate([r["out"] for r in res.results], axis=0)  # [B,128,OW]
    sume = np.concatenate(
        [r["sumd"].reshape(BPC, O) for r in res.results], axis=0
    )  # [B, O]
    full = raw.astype(np.float32).reshape(B, TCH, NTH, O) / sume[:, None, None, :]
    return np.ascontiguousarray(full.transpose(0, 3, 2, 1).reshape(B, O, T))
